# revision 4
# baseline (speedup 1.0000x reference)
"""Decorrelated BatchNorm (ZCA whitening) Trainium2 kernel, 8-core SPMD.

Strategy (data-parallel over batch, per sharding hint):
  - Each core gets 1/8 of the rows (batch*H*W) of x [131072, 256].
  - Phase A (streamed over 128 row-tiles of [128, 256]):
      * per-chunk (2 chunks of 128 channels) Gram matrix X_k^T X_k and
        channel sums via PE matmuls accumulated in PSUM
      * PE-transpose of each tile chunk -> X^T kept resident in SBUF
  - AllReduce (8 cores) of [Gram | sums] (132 KB).
  - Phase B (tiny, redundant on every core): sigma = G/n - mu mu^T + eps I,
    masked to the 16x16 block-diagonal; coupled Newton-Schulz iteration
    gives Z = sigma^(-1/2) (block-diagonal, symmetric); fold gamma into
    W = Z * diag(gamma); bias b = beta - mu @ W  (folds the centering).
  - Phase C: out = X @ W + b via PE matmuls (lhsT = resident X^T), bias
    added during PSUM->SBUF eviction, streamed DMA out.
"""

import numpy as np

import concourse.bass as bass
import concourse.tile as tile
from concourse import mybir
from concourse.bass_utils import run_bass_kernel_spmd

F32 = mybir.dt.float32
NCORES = 8
N, H, Wd, C = 128, 32, 32, 256
P = 128                 # partitions
R = (N * H * Wd) // NCORES   # rows per core = 16384
TILES = R // P          # 128
NTOT = N * H * Wd       # 131072
M_GROUP = 16            # channels per whitening group
EPS = 1e-5
NS_ITERS = 5


def _split_multi_waits(nc):
    """Walrus in this container accepts at most one attached sync wait per
    instruction; hoist extras into standalone wait instructions."""
    for f in nc.m.functions:
        for bb in f.blocks:
            new_insts = []
            for ins in bb.instructions:
                si = ins.sync_info
                if si is not None and len(si.on_wait) > 1:
                    waits = list(si.on_wait)
                    for j, w in enumerate(waits[:-1]):
                        nop = mybir.InstEventSemaphore(
                            name=f"{ins.name}-wsplit{j}", ins=[], outs=[])
                        nop.engine = ins.engine
                        nop.sync_info = mybir.SyncInfo(on_wait=[w], on_update=[])
                        nc.register_instruction(nop)
                        new_insts.append(nop)
                    ins.sync_info = mybir.SyncInfo(
                        on_wait=[waits[-1]], on_update=list(si.on_update))
                new_insts.append(ins)
            bb.instructions = new_insts


def _bcast_ap(handle, parts, cols, offset=0):
    """DRAM AP replicating a [cols] vector across `parts` partitions."""
    return bass.AP(tensor=handle, offset=offset, ap=[[0, parts], [1, cols]])


def build_program():
    nc = bass.Bass(num_devices=NCORES)
    x = nc.declare_dram_parameter("x_shard", [R, C], F32, isOutput=False)
    gamma = nc.declare_dram_parameter("gamma", [C], F32, isOutput=False)
    beta = nc.declare_dram_parameter("beta", [C], F32, isOutput=False)
    out = nc.declare_dram_parameter("out_shard", [R, C], F32, isOutput=True)

    cc_in = nc.dram_tensor("cc_in", [P, 258], F32)
    cc_out = nc.dram_tensor("cc_out", [P, 258], F32, addr_space="Shared")
    b_dram = nc.dram_tensor("b_dram", [C], F32)

    eye = np.eye(P, dtype=np.float32)
    blockmask = np.kron(np.eye(P // M_GROUP, dtype=np.float32),
                        np.ones((M_GROUP, M_GROUP), np.float32))
    ident_t = nc.inline_tensor(eye, name="identc")
    i15_t = nc.inline_tensor(1.5 * eye, name="i15c")
    epsi_t = nc.inline_tensor(EPS * eye, name="epsic")
    mask_t = nc.inline_tensor(blockmask, name="maskc")
    ones_t = nc.inline_tensor(np.ones((P, 1), np.float32), name="onesc")

    inv_n = 1.0 / NTOT

    with tile.TileContext(nc) as tc:
        with tc.tile_pool(name="consts", bufs=1) as consts:
            sb_ident = consts.tile([P, P], F32, tag="ident")
            nc.sync.dma_start(out=sb_ident, in_=ident_t[:])
            sb_i15 = consts.tile([P, P], F32, tag="i15")
            nc.sync.dma_start(out=sb_i15, in_=i15_t[:])
            sb_epsi = consts.tile([P, P], F32, tag="epsi")
            nc.sync.dma_start(out=sb_epsi, in_=epsi_t[:])
            sb_mask = consts.tile([P, P], F32, tag="mask")
            nc.sync.dma_start(out=sb_mask, in_=mask_t[:])
            sb_ones = consts.tile([P, 1], F32, tag="ones")
            nc.sync.dma_start(out=sb_ones, in_=ones_t[:])
            sb_grep = consts.tile([P, C], F32, tag="grep")
            nc.sync.dma_start(out=sb_grep, in_=_bcast_ap(gamma, P, C))
            sb_brow = consts.tile([1, C], F32, tag="brow")
            nc.sync.dma_start(out=sb_brow, in_=beta[:].unsqueeze(0))

            # resident transposed shard: xt[p=channel-in-chunk, k=chunk, r=row]
            xt = consts.tile([P, 2, R], F32, tag="xt")

            # ---------- Phase A: stats + transpose, streamed ----------
            # NOTE: each accumulating matmul needs its OWN psum tile (bank) —
            # two interleaved fp32 accumulation groups sharing a bank corrupt
            # each other on HW (measured: 1% error on the Gram diagonal).
            with (tc.tile_pool(name="pstats", bufs=1, space="PSUM") as pstats,
                  tc.tile_pool(name="xin", bufs=4) as xpool,
                  tc.tile_pool(name="tpsum", bufs=4, space="PSUM") as tpsum):
                ps_g = [pstats.tile([P, P], F32, tag="g0", name="psg0"),
                        pstats.tile([P, P], F32, tag="g1", name="psg1")]
                ps_s = [pstats.tile([P, 1], F32, tag="su0", name="pss0"),
                        pstats.tile([P, 1], F32, tag="su1", name="pss1")]
                for t in range(TILES):
                    xt_tile = xpool.tile([P, C], F32, tag="xtile")
                    nc.sync.dma_start(out=xt_tile, in_=x[t * P:(t + 1) * P, :])
                    flags = dict(start=(t == 0), stop=(t == TILES - 1))
                    for k in range(2):
                        ch = slice(k * P, (k + 1) * P)
                        nc.tensor.matmul(ps_g[k], lhsT=xt_tile[:, ch],
                                         rhs=xt_tile[:, ch], **flags)
                        nc.tensor.matmul(ps_s[k], lhsT=xt_tile[:, ch],
                                         rhs=sb_ones, **flags)
                        pt = tpsum.tile([P, P], F32, tag="pt")
                        nc.tensor.transpose(pt, xt_tile[:, ch], sb_ident)
                        nc.vector.tensor_copy(xt[:, k, t * P:(t + 1) * P], pt)

                # evict stats to SBUF: [G0 | s0 | G1 | s1] = cols
                # [0:128 | 128 | 129:257 | 257]
                st_loc = consts.tile([P, 258], F32, tag="stloc")
                nc.vector.tensor_copy(st_loc[:, 0:128], ps_g[0])
                nc.vector.tensor_copy(st_loc[:, 128:129], ps_s[0])
                nc.vector.tensor_copy(st_loc[:, 129:257], ps_g[1])
                nc.vector.tensor_copy(st_loc[:, 257:258], ps_s[1])

            # ---------- AllReduce ----------
            nc.sync.dma_start(out=cc_in[:], in_=st_loc)
            nc.gpsimd.collective_compute(
                "AllReduce", mybir.AluOpType.add,
                replica_groups=[list(range(NCORES))],
                ins=[cc_in[:]], outs=[cc_out[:]],
            )
            st = consts.tile([P, 258], F32, tag="stglob")
            nc.sync.dma_start(out=st, in_=cc_out[:])

            # ---------- Phase B: sigma, Newton-Schulz, W and bias ----------
            w_tiles = []
            with (tc.tile_pool(name="small", bufs=1) as spool,
                  tc.tile_pool(name="npsum", bufs=3, space="PSUM") as npsum):
                # mu columns: sums are at st cols 128 (chunk0) and 257 (chunk1)
                mu = spool.tile([P, 2], F32, tag="mu")
                nc.vector.tensor_scalar_mul(mu[:, 0:1], st[:, 128:129], inv_n)
                nc.vector.tensor_scalar_mul(mu[:, 1:2], st[:, 257:258], inv_n)

                murow = []
                for k in range(2):
                    pmu = npsum.tile([1, P], F32, tag="ns")
                    nc.tensor.transpose(pmu, mu[:, k:k + 1], sb_ident)
                    mr = spool.tile([1, P], F32, tag=f"murow{k}")
                    nc.vector.tensor_copy(mr, pmu)
                    murow.append(mr)

                b_row = spool.tile([1, C], F32, tag="bias_row")
                for k in range(2):
                    gram_cols = slice(0, 128) if k == 0 else slice(129, 257)
                    pouter = npsum.tile([P, P], F32, tag="ns")
                    nc.tensor.matmul(pouter, lhsT=murow[k], rhs=murow[k])
                    sg = spool.tile([P, P], F32, tag=f"sig{k}")
                    nc.vector.tensor_scalar_mul(sg, st[:, gram_cols], inv_n)
                    nc.vector.tensor_sub(sg, sg, pouter)
                    nc.vector.tensor_mul(sg, sg, sb_mask)
                    nc.vector.tensor_add(sg, sg, sb_epsi)

                    # coupled Newton-Schulz: Y->sigma^(1/2), Z->sigma^(-1/2)
                    z = spool.tile([P, P], F32, tag=f"z{k}")
                    nc.vector.tensor_copy(z, sb_ident)
                    y = sg
                    for _ in range(NS_ITERS):
                        pT = npsum.tile([P, P], F32, tag="ns")
                        nc.tensor.matmul(pT, lhsT=z, rhs=y)
                        b = spool.tile([P, P], F32, tag=f"nsb{k}")
                        nc.vector.tensor_scalar_mul(b, pT, -0.5)
                        nc.vector.tensor_add(b, b, sb_i15)
                        pY = npsum.tile([P, P], F32, tag="ns")
                        nc.tensor.matmul(pY, lhsT=y, rhs=b)
                        nc.vector.tensor_copy(y, pY)
                        pZ = npsum.tile([P, P], F32, tag="ns")
                        nc.tensor.matmul(pZ, lhsT=b, rhs=z)
                        nc.vector.tensor_copy(z, pZ)

                    # fold gamma: W = Z * diag(gamma_chunk)
                    wk = consts.tile([P, P], F32, tag=f"w{k}")
                    nc.vector.tensor_mul(wk, z, sb_grep[:, k * P:(k + 1) * P])
                    w_tiles.append(wk)

                    # bias chunk: beta - mu @ W
                    pb = npsum.tile([1, P], F32, tag="ns")
                    nc.tensor.matmul(pb, lhsT=mu[:, k:k + 1], rhs=wk)
                    nc.vector.tensor_sub(b_row[:, k * P:(k + 1) * P],
                                         sb_brow[:, k * P:(k + 1) * P], pb)

                # replicate bias across partitions via DRAM bounce
                nc.sync.dma_start(out=b_dram[:], in_=b_row)
                b_rep = consts.tile([P, C], F32, tag="brep")
                nc.sync.dma_start(out=b_rep, in_=_bcast_ap(b_dram, P, C))

            # ---------- Phase C: whiten + bias + store ----------
            with (tc.tile_pool(name="opsum", bufs=4, space="PSUM") as opsum,
                  tc.tile_pool(name="osb", bufs=4) as opool):
                for t in range(TILES):
                    o_sb = opool.tile([P, C], F32, tag="osb")
                    for k in range(2):
                        po = opsum.tile([P, P], F32, tag="po")
                        nc.tensor.matmul(po, lhsT=xt[:, k, t * P:(t + 1) * P],
                                         rhs=w_tiles[k])
                        nc.vector.tensor_add(o_sb[:, k * P:(k + 1) * P], po,
                                             b_rep[:, k * P:(k + 1) * P])
                    nc.sync.dma_start(out=out[t * P:(t + 1) * P, :], in_=o_sb)

    _split_multi_waits(nc)
    return nc


_NC_CACHE = None


def _get_program():
    global _NC_CACHE
    if _NC_CACHE is None:
        _NC_CACHE = build_program()
    return _NC_CACHE


def run(x, gamma, beta, trace=False):
    """Run the SPMD kernel on full inputs; returns (output, BassKernelResults)."""
    x = np.ascontiguousarray(np.asarray(x, np.float32))
    gamma = np.ascontiguousarray(np.asarray(gamma, np.float32).reshape(C))
    beta = np.ascontiguousarray(np.asarray(beta, np.float32).reshape(C))
    xf = x.reshape(NTOT, C)
    nc = _get_program()
    in_maps = [
        {"x_shard": np.ascontiguousarray(xf[i * R:(i + 1) * R]),
         "gamma": gamma, "beta": beta}
        for i in range(NCORES)
    ]
    res = run_bass_kernel_spmd(nc, in_maps, core_ids=list(range(NCORES)),
                               trace=trace)
    outf = np.concatenate([res.results[i]["out_shard"] for i in range(NCORES)],
                          axis=0)
    return outf.reshape(N, H, Wd, C), res


def kernel(x, gamma, beta):
    out, _ = run(x, gamma, beta)
    return out


# revision 6
# speedup vs baseline: 1.3652x; 1.3652x over previous
"""Decorrelated BatchNorm (ZCA whitening) Trainium2 kernel, 8-core SPMD.

Strategy (data-parallel over batch, per sharding hint):
  - Each core gets 1/8 of the rows (batch*H*W) of x [131072, 256].
  - Phase A (streamed over 128 row-tiles of [128, 256]):
      * fp32r matmul per channel-chunk: [Gram | sums] in one accumulating
        matmul (ones column appended to the moving operand)
      * fp32 PE-transpose of each tile chunk -> X^T kept resident in SBUF
        (stored as fp32r by the PSUM->SBUF eviction copy)
  - AllReduce (8 cores) of [Gram | sums] (132 KB).
  - Phase B (tiny, redundant on every core): sigma = G/n - mu mu^T + eps I,
    masked to the 16x16 block-diagonal; coupled Newton-Schulz iteration
    gives Z = sigma^(-1/2); fold gamma into W = Z * diag(gamma); bias
    b = beta - mu @ W (folds the centering).
  - Phase C: out = X @ W + b, two fp32r matmuls (N=256, zero-padded W
    halves) accumulating into one PSUM tile; bias added during eviction.

HW notes baked in:
  - walrus here allows ONE attached sync wait per instruction -> extra
    waits are split into standalone InstEventSemaphore (see
    _split_multi_waits).
  - two interleaved fp32 accumulation groups sharing a PSUM bank corrupt
    each other -> one accumulating matmul per PSUM tile.
  - fp32r matmul operands must be produced by an instruction that rounds
    to fp32r (BIR verifier) -> DVE/Pool copies into fp32r tiles.
"""

import numpy as np

import concourse.bass as bass
import concourse.tile as tile
from concourse import mybir
from concourse.bass_utils import run_bass_kernel_spmd

F32 = mybir.dt.float32
F32R = mybir.dt.float32r
NCORES = 8
N, H, Wd, C = 128, 32, 32, 256
P = 128                      # partitions
R = (N * H * Wd) // NCORES   # rows per core = 16384
TILES = R // P               # 128
NTOT = N * H * Wd            # 131072
M_GROUP = 16                 # channels per whitening group
EPS = 1e-5
NS_ITERS = 4


def _split_multi_waits(nc):
    for f in nc.m.functions:
        for bb in f.blocks:
            new_insts = []
            for ins in bb.instructions:
                si = ins.sync_info
                if si is not None and len(si.on_wait) > 1:
                    waits = list(si.on_wait)
                    for j, w in enumerate(waits[:-1]):
                        nop = mybir.InstEventSemaphore(
                            name=f"{ins.name}-wsplit{j}", ins=[], outs=[])
                        nop.engine = ins.engine
                        nop.sync_info = mybir.SyncInfo(on_wait=[w], on_update=[])
                        nc.register_instruction(nop)
                        new_insts.append(nop)
                    ins.sync_info = mybir.SyncInfo(
                        on_wait=[waits[-1]], on_update=list(si.on_update))
                new_insts.append(ins)
            bb.instructions = new_insts


def _bcast_ap(handle, parts, cols, offset=0):
    """DRAM AP replicating a [cols] vector across `parts` partitions."""
    return bass.AP(tensor=handle, offset=offset, ap=[[0, parts], [1, cols]])


def build_program():
    nc = bass.Bass(num_devices=NCORES)
    x = nc.declare_dram_parameter("x_shard", [R, C], F32, isOutput=False)
    gamma = nc.declare_dram_parameter("gamma", [C], F32, isOutput=False)
    beta = nc.declare_dram_parameter("beta", [C], F32, isOutput=False)
    out = nc.declare_dram_parameter("out_shard", [R, C], F32, isOutput=True)

    cc_in = nc.dram_tensor("cc_in", [P, 258], F32)
    cc_out = nc.dram_tensor("cc_out", [P, 258], F32, addr_space="Shared")
    b_dram = nc.dram_tensor("b_dram", [C], F32)

    eye = np.eye(P, dtype=np.float32)
    blockmask = np.kron(np.eye(P // M_GROUP, dtype=np.float32),
                        np.ones((M_GROUP, M_GROUP), np.float32))
    ident_t = nc.inline_tensor(eye, name="identc")
    i15_t = nc.inline_tensor(1.5 * eye, name="i15c")
    epsi_t = nc.inline_tensor(EPS * eye, name="epsic")
    mask_t = nc.inline_tensor(blockmask, name="maskc")

    inv_n = 1.0 / NTOT

    with tile.TileContext(nc) as tc:
        with tc.tile_pool(name="consts", bufs=1) as consts:
            sb_ident = consts.tile([P, P], F32, tag="ident")
            nc.sync.dma_start(out=sb_ident, in_=ident_t[:])
            sb_i15 = consts.tile([P, P], F32, tag="i15")
            nc.sync.dma_start(out=sb_i15, in_=i15_t[:])
            sb_epsi = consts.tile([P, P], F32, tag="epsi")
            nc.sync.dma_start(out=sb_epsi, in_=epsi_t[:])
            sb_mask = consts.tile([P, P], F32, tag="mask")
            nc.sync.dma_start(out=sb_mask, in_=mask_t[:])
            sb_grep = consts.tile([P, C], F32, tag="grep")
            nc.sync.dma_start(out=sb_grep, in_=_bcast_ap(gamma, P, C))
            sb_brow = consts.tile([1, C], F32, tag="brow")
            nc.sync.dma_start(out=sb_brow, in_=beta[:].unsqueeze(0))

            # resident transposed shard (fp32r): xt[ch-in-chunk, chunk, row]
            xt = consts.tile([P, 2, R], F32R, tag="xt")

            # ---------- Phase A: stats + transpose, streamed ----------
            with (tc.tile_pool(name="pstats", bufs=1, space="PSUM") as pstats,
                  tc.tile_pool(name="xin", bufs=4) as xpool,
                  tc.tile_pool(name="tpsum", bufs=4, space="PSUM") as tpsum):
                ps_g = [pstats.tile([P, 258], F32, tag="g0", name="psg0"),
                        pstats.tile([P, 258], F32, tag="g1", name="psg1")]
                for t in range(TILES):
                    x_tile = xpool.tile([P, 258], F32, tag="xtile")
                    nc.sync.dma_start(out=x_tile[:, 0:256],
                                      in_=x[t * P:(t + 1) * P, :])
                    # two ones columns: fp32r matmul needs even free size
                    nc.gpsimd.memset(x_tile[:, 256:258], 1.0)
                    # fp32r-rounded copy (Pool engine; DVE is busier)
                    x_r = xpool.tile([P, 258], F32R, tag="xr")
                    nc.gpsimd.tensor_copy(x_r, x_tile)
                    flags = dict(start=(t == 0), stop=(t == TILES - 1))
                    for k in range(2):
                        ch = slice(k * P, (k + 1) * P)
                        nc.tensor.matmul(ps_g[k], lhsT=x_r[:, ch],
                                         rhs=x_r, **flags)
                        pt = tpsum.tile([P, P], F32, tag="pt")
                        nc.tensor.transpose(pt, x_tile[:, ch], sb_ident)
                        nc.vector.tensor_copy(xt[:, k, t * P:(t + 1) * P], pt)

                # evict stats: st_loc = [G0 | s0 | G1 | s1]
                st_loc = consts.tile([P, 258], F32, tag="stloc")
                nc.vector.tensor_copy(st_loc[:, 0:128], ps_g[0][:, 0:128])
                nc.vector.tensor_copy(st_loc[:, 128:129], ps_g[0][:, 256:257])
                nc.vector.tensor_copy(st_loc[:, 129:257], ps_g[1][:, 128:256])
                nc.vector.tensor_copy(st_loc[:, 257:258], ps_g[1][:, 256:257])

            # ---------- AllReduce ----------
            nc.sync.dma_start(out=cc_in[:], in_=st_loc)
            nc.gpsimd.collective_compute(
                "AllReduce", mybir.AluOpType.add,
                replica_groups=[list(range(NCORES))],
                ins=[cc_in[:]], outs=[cc_out[:]],
            )
            st = consts.tile([P, 258], F32, tag="stglob")
            nc.sync.dma_start(out=st, in_=cc_out[:])

            # ---------- Phase B: sigma, Newton-Schulz, W and bias ----------
            wpad = []
            with (tc.tile_pool(name="small", bufs=1) as spool,
                  tc.tile_pool(name="npsum", bufs=4, space="PSUM") as npsum):
                mu = spool.tile([P, 2], F32, tag="mu")
                nc.vector.tensor_scalar_mul(mu[:, 0:1], st[:, 128:129], inv_n)
                nc.vector.tensor_scalar_mul(mu[:, 1:2], st[:, 257:258], inv_n)

                murow, sig, zs, bs = [], [], [], []
                for k in range(2):
                    pmu = npsum.tile([1, P], F32, tag="ns", name=f"pmu{k}")
                    nc.tensor.transpose(pmu, mu[:, k:k + 1], sb_ident)
                    mr = spool.tile([1, P], F32, tag=f"murow{k}",
                                    name=f"murow{k}")
                    nc.vector.tensor_copy(mr, pmu)
                    murow.append(mr)

                for k in range(2):
                    gram_cols = slice(0, 128) if k == 0 else slice(129, 257)
                    pouter = npsum.tile([P, P], F32, tag="ns",
                                        name=f"pout{k}")
                    nc.tensor.matmul(pouter, lhsT=murow[k], rhs=murow[k])
                    sg = spool.tile([P, P], F32, tag=f"sig{k}", name=f"sig{k}")
                    nc.vector.tensor_scalar_mul(sg, st[:, gram_cols], inv_n)
                    nc.vector.tensor_sub(sg, sg, pouter)
                    nc.vector.tensor_mul(sg, sg, sb_mask)
                    nc.vector.tensor_add(sg, sg, sb_epsi)
                    sig.append(sg)
                    z = spool.tile([P, P], F32, tag=f"z{k}", name=f"z{k}")
                    nc.vector.tensor_copy(z, sb_ident)
                    zs.append(z)
                    bs.append(spool.tile([P, P], F32, tag=f"nsb{k}",
                                         name=f"nsb{k}"))

                # coupled Newton-Schulz, both chunks interleaved
                for _ in range(NS_ITERS):
                    for k in range(2):
                        pT = npsum.tile([P, P], F32, tag="ns", name="pT")
                        nc.tensor.matmul(pT, lhsT=zs[k], rhs=sig[k])
                        nc.vector.tensor_scalar_mul(bs[k], pT, -0.5)
                        nc.vector.tensor_add(bs[k], bs[k], sb_i15)
                    for k in range(2):
                        pY = npsum.tile([P, P], F32, tag="ns", name="pY")
                        nc.tensor.matmul(pY, lhsT=sig[k], rhs=bs[k])
                        nc.vector.tensor_copy(sig[k], pY)
                        pZ = npsum.tile([P, P], F32, tag="ns", name="pZ")
                        nc.tensor.matmul(pZ, lhsT=bs[k], rhs=zs[k])
                        nc.vector.tensor_copy(zs[k], pZ)

                b_row = spool.tile([1, C], F32, tag="bias_row")
                for k in range(2):
                    ch = slice(k * P, (k + 1) * P)
                    # staged fp32 W half: [0 | Z_k * diag(gamma_k) | 0]
                    wst = spool.tile([P, C], F32, tag=f"wst{k}", name=f"wst{k}")
                    nc.gpsimd.memset(wst, 0.0)
                    nc.vector.tensor_mul(wst[:, ch], zs[k], sb_grep[:, ch])
                    wp = consts.tile([P, C], F32R, tag=f"wpad{k}",
                                     name=f"wpad{k}")
                    nc.vector.tensor_copy(wp, wst)
                    wpad.append(wp)
                    # bias chunk: beta - mu @ W
                    pb = npsum.tile([1, P], F32, tag="ns", name=f"pb{k}")
                    nc.tensor.matmul(pb, lhsT=mu[:, k:k + 1], rhs=wst[:, ch])
                    nc.vector.tensor_sub(b_row[:, ch], sb_brow[:, ch], pb)

                # replicate bias across partitions via DRAM bounce
                nc.sync.dma_start(out=b_dram[:], in_=b_row)
                b_rep = consts.tile([P, C], F32, tag="brep")
                nc.sync.dma_start(out=b_rep, in_=_bcast_ap(b_dram, P, C))

            # ---------- Phase C: whiten + bias + store ----------
            with (tc.tile_pool(name="opsum", bufs=4, space="PSUM") as opsum,
                  tc.tile_pool(name="osb", bufs=4) as opool):
                for t in range(TILES):
                    po = opsum.tile([P, C], F32, tag="po")
                    rows = slice(t * P, (t + 1) * P)
                    nc.tensor.matmul(po, lhsT=xt[:, 0, rows], rhs=wpad[0],
                                     start=True, stop=False)
                    nc.tensor.matmul(po, lhsT=xt[:, 1, rows], rhs=wpad[1],
                                     start=False, stop=True)
                    o_sb = opool.tile([P, C], F32, tag="osb")
                    nc.vector.tensor_add(o_sb, po, b_rep)
                    nc.sync.dma_start(out=out[t * P:(t + 1) * P, :], in_=o_sb)

    _split_multi_waits(nc)
    return nc


_NC_CACHE = None


def _get_program():
    global _NC_CACHE
    if _NC_CACHE is None:
        _NC_CACHE = build_program()
    return _NC_CACHE


def run(x, gamma, beta, trace=False):
    """Run the SPMD kernel on full inputs; returns (output, BassKernelResults)."""
    x = np.ascontiguousarray(np.asarray(x, np.float32))
    gamma = np.ascontiguousarray(np.asarray(gamma, np.float32).reshape(C))
    beta = np.ascontiguousarray(np.asarray(beta, np.float32).reshape(C))
    xf = x.reshape(NTOT, C)
    nc = _get_program()
    in_maps = [
        {"x_shard": np.ascontiguousarray(xf[i * R:(i + 1) * R]),
         "gamma": gamma, "beta": beta}
        for i in range(NCORES)
    ]
    res = run_bass_kernel_spmd(nc, in_maps, core_ids=list(range(NCORES)),
                               trace=trace)
    outf = np.concatenate([res.results[i]["out_shard"] for i in range(NCORES)],
                          axis=0)
    return outf.reshape(N, H, Wd, C), res


def kernel(x, gamma, beta):
    out, _ = run(x, gamma, beta)
    return out
